# revision 19
# baseline (speedup 1.0000x reference)
"""Trainium2 Bass kernel for quantized int8 per-channel Conv2d.

Reference semantics (fp32):
  x_f = (x_int8 - 7) * 0.01                      # per-tensor dequant
  w_f = (w_int8 - zp[cout]) * scale[cout]        # per-channel dequant
  y   = round(conv2d_valid(x_f, w_f) + bias[cout])  -> int32

Algorithm: 1D Winograd F(2,3) along the width axis, direct 3-tap
accumulation along height.  Per 2 output columns the width conv needs 4
matmul points instead of 6 -> PE work drops 1.5x vs direct conv:

  y[h, 2p:2p+2] = A^T [ (G w_row) * (B^T d) ],  d = x[h, 2p:2p+4]
  B^T d = [d0-d2, d1+d2, d2-d1, d1-d3]   (all +-1 -> plain tensor_tensor)
  y0 = m0+m1+m2,  y1 = m1-m2-m3

Matmul operands are fp16: transformed inputs are integers |V| <= 256,
exact in fp16; U = G*(w-zp)*0.01*scale carries 2^-11 relative error.
Accumulation is fp32 in PSUM.  Host folds the (x-7) per-tensor zeropoint
into the bias via conv(x-7,wq) = conv(x,wq) - 7*sum(wq).

Sharding: data-parallel over batch N=32 across 8 cores (4 images each);
weights/bias replicated.

Engine split: PE matmuls (point-major, 3 row-chunks innermost for x3
weight reuse, one PSUM bank per point/chunk); ACT drains PSUM to SBUF
fp16, adding the bias on the M1 point (coefficient +1 in both outputs);
DVE does the stride-2 "deal" unzip, half the transforms, the A^T
combines (packed fp16, 2x mode) and the magic-number int32 rounding;
GpSimd takes the other half of the transforms.
"""

import numpy as np

import concourse.bass as bass
import concourse.mybir as mybir
from concourse import bacc
from concourse.tile import TileContext
from concourse.bass_utils import run_bass_kernel_spmd

# Problem shapes (hardcoded per contract)
N, CIN, H, W = 32, 256, 56, 56
COUT, KH, KW = 256, 3, 3
HO, WO = H - KH + 1, W - KW + 1          # 54, 54
NCORES = 8
NPER = N // NCORES                        # images per core
HW = H * W                                # 3136
XPAD = HW + 64
KT = CIN // 128                           # 2 cin tiles
MT = COUT // 128                          # 2 cout tiles
NJ = 4                                    # F(2,3) winograd points
NP = 28                                   # col half-pairs per row (27 used)
TP = H * NP                               # 1568 transform cols per point
TP2 = TP + 2                              # deal plane width (shift slack)
CH = 18                                   # output rows per chunk
NCH = 3                                   # chunks per (img, m)
NCOLS = CH * 27                           # 486 matmul free dim
MAGIC = 12582912.0                        # 1.5 * 2**23 fp32 RNE round trick

GMAT = np.array([
    [1, 0, 0],
    [0.5, 0.5, 0.5],
    [0.5, -0.5, 0.5],
    [0, 0, 1],
], dtype=np.float64)

_CACHE = {}


def _build_program():
    nc = bacc.Bacc("TRN2", target_bir_lowering=False, debug=False,
                   num_devices=NCORES)
    dt = mybir.dt
    f16 = dt.float16
    AF = mybir.ActivationFunctionType
    ALU = mybir.AluOpType

    x_d = nc.dram_tensor("x", [NPER, CIN, H, W], dt.int8, kind="ExternalInput")
    # U layout: [k, cin_part, j, r, m, cout_part]
    u_d = nc.dram_tensor("u", [KT, 128, NJ, KH, MT, 128], f16,
                         kind="ExternalInput")
    b2_d = nc.dram_tensor("bias2", [COUT], dt.float32, kind="ExternalInput")
    out_d = nc.dram_tensor("out", [NPER, COUT, HO, WO], dt.int32,
                           kind="ExternalOutput")

    with TileContext(nc) as tc:
        with (
            tc.tile_pool(name="const", bufs=1) as cpool,
            tc.tile_pool(name="xin", bufs=2) as xpool,
            tc.tile_pool(name="xq", bufs=2) as qpool,
            tc.tile_pool(name="vbuf", bufs=3) as vpool,
            tc.tile_pool(name="psum", bufs=7, space="PSUM") as ppool,
            tc.tile_pool(name="msb", bufs=2) as mpool,
            tc.tile_pool(name="csc", bufs=4) as epool,
            tc.tile_pool(name="yb", bufs=4) as ypool,
            tc.tile_pool(name="outb", bufs=2) as opool,
        ):
            # ---- constants ----
            usb = cpool.tile([128, KT, NJ, KH, MT, 128], f16)
            for k in range(KT):
                nc.sync.dma_start(out=usb[:, k], in_=u_d[k])
            b2 = cpool.tile([128, MT], dt.float32)
            nc.sync.dma_start(out=b2[:, :],
                              in_=b2_d.rearrange("(m p) -> p m", p=128))

            # PE warm-up: SUBSTANTIVE matmuls (full 128-wide array,
            # 512-col rhs) so the HAM activity monitor actually sees the
            # PE busy and lifts the clock gate to 8/8 before the first
            # real matmul.  Tiny 1-partition warmups do not register.
            wupw = cpool.tile([128, 128], f16)
            nc.vector.memset(wupw[:, :], 1.0)
            wupx = cpool.tile([128, 512], f16)
            nc.vector.memset(wupx[:, :], 1.0)
            wups = ppool.tile([128, 512], dt.float32, name="wups", tag="wup",
                              bufs=1)
            for _ in range(12):
                nc.tensor.matmul(wups[:, :], wupw[:, :], wupx[:, :],
                                 start=True, stop=True)

            def keepers(rhs, count):
                # matmuls gated on `rhs` readiness: keep the PE's HAM
                # activity window busy while startup DMA/transforms land
                for _ in range(count):
                    nc.tensor.matmul(wups[:, 0:rhs.shape[-1]], wupw[:, :],
                                     rhs, start=True, stop=True)

            def xdma(n, xb):
                for k in range(KT):
                    nc.gpsimd.dma_start(
                        out=xb[:, k, 0:HW],
                        in_=x_d[n, k * 128:(k + 1) * 128].rearrange(
                            "p h w -> p (h w)"))

            def transform(xb, vt, split_k):
                # V ops read strided int8 views of x directly (strides do
                # not add DVE cycles at 1x): d_j[t] = x[2t + j], t = 28h+p
                if split_k:
                    # k-interleaved, j-major: matmul group j needs only
                    # V_j of both k, ready after 2 ops
                    def dj(j, k):
                        return xb[:, k, j:j + 2 * TP].rearrange(
                            "p (t f) -> p t f", f=2)[:, :, 0]
                    for j, (a, b, op) in enumerate(
                            [(0, 2, ALU.subtract), (1, 2, ALU.add),
                             (2, 1, ALU.subtract), (1, 3, ALU.subtract)]):
                        for k in range(KT):
                            eng = nc.gpsimd if j == 3 else nc.vector
                            eng.tensor_tensor(
                                vt[:, k, j], dj(a, k), dj(b, k), op)
                else:
                    def dj(j):
                        return xb[:, :, j:j + 2 * TP].rearrange(
                            "p k (t f) -> p k t f", f=2)[:, :, :, 0]
                    d0, d1, d2, d3 = (dj(j) for j in range(4))
                    v = lambda j: vt[:, :, j]
                    nc.vector.tensor_tensor(v(0), d0, d2, ALU.subtract)
                    nc.vector.tensor_tensor(v(1), d1, d2, ALU.add)
                    nc.vector.tensor_tensor(v(2), d2, d1, ALU.subtract)
                    nc.vector.tensor_tensor(v(3), d1, d3, ALU.subtract)

            # ---- image 0 load + transform ----
            xb0 = xpool.tile([128, KT, XPAD], dt.int8, name="xb")
            vt0 = vpool.tile([128, KT, NJ, TP], f16, name="vt")
            xdma(0, xb0)
            keepers(usb[:, 0, 0, 0, 0, :], 8)
            transform(xb0, vt0, split_k=True)
            keepers(vt0[:, 0, 0, 0:512], 6)

            vts = [vt0]
            for n in range(NPER):
                vt = vts[n]
                if n + 1 < NPER:
                    xbn = xpool.tile([128, KT, XPAD], dt.int8, name="xb")
                    vtn = vpool.tile([128, KT, NJ, TP], f16, name="vt")
                    xdma(n + 1, xbn)
                    transform(xbn, vtn, split_k=False)
                    vts.append(vtn)

                for m in range(MT):
                    msb = mpool.tile([128, NJ, NCH * NCOLS], f16, name="msb")
                    for j in range(NJ):
                        ps = [ppool.tile([128, NCOLS], dt.float32,
                                         name="ps", tag="ps")
                              for _ in range(NCH)]
                        for k in range(KT):
                            for r in range(KH):
                                lhsT = usb[:, k, j, r, m, :]
                                for c in range(NCH):
                                    nc.tensor.matmul(
                                        ps[c][:, :], lhsT,
                                        vt[:, k, j].rearrange(
                                            "p (h q) -> p h q", q=NP)[
                                            :, CH * c + r:CH * c + r + CH,
                                            0:27],
                                        start=(r == 0 and k == 0),
                                        stop=(r == KH - 1 and k == KT - 1))
                        # drain point j to SBUF fp16; point 1 enters both
                        # outputs with coefficient +1 -> carries the bias.
                        tail = (n == NPER - 1 and m == MT - 1 and j >= 2)
                        for c in range(NCH):
                            dst = msb[:, j, NCOLS * c:NCOLS * (c + 1)]
                            if j == 1:
                                nc.scalar.activation(
                                    dst, ps[c][:, :], AF.Identity,
                                    bias=b2[:, m:m + 1], scale=1.0)
                            elif tail:
                                nc.vector.tensor_scalar(
                                    dst, ps[c][:, :], 0.0, None, ALU.add)
                            else:
                                nc.scalar.activation(dst, ps[c][:, :],
                                                     AF.Copy)

                    # ---- A^T combines (DVE) + rounding + assemble ----
                    # final (img, m): per-chunk epilogue to cut the tail
                    # latency after the last matmul; otherwise one pass
                    ob = opool.tile([128, HO, 27, 2], dt.int32, name="ob")
                    last = (n == NPER - 1 and m == MT - 1)
                    chgrp = range(NCH) if last else [slice(None)]
                    for cc in chgrp:
                        sl = (slice(NCOLS * cc, NCOLS * (cc + 1))
                              if last else slice(None))
                        nch = 1 if last else NCH
                        M = [msb[:, j, sl] for j in range(NJ)]
                        t0 = epool.tile([128, NCH * NCOLS], f16,
                                        name="c0")[:, sl if last
                                                   else slice(None)]
                        t1 = epool.tile([128, NCH * NCOLS], f16,
                                        name="c1")[:, sl if last
                                                   else slice(None)]
                        y0 = ypool.tile([128, NCH * NCOLS], f16,
                                        name="y0")[:, sl if last
                                                   else slice(None)]
                        y1 = ypool.tile([128, NCH * NCOLS], f16,
                                        name="y1")[:, sl if last
                                                   else slice(None)]
                        nc.vector.tensor_tensor(t0, M[0], M[1], ALU.add)
                        nc.vector.tensor_tensor(y0, t0, M[2], ALU.add)
                        nc.vector.tensor_tensor(t1, M[1], M[2], ALU.subtract)
                        nc.vector.tensor_tensor(y1, t1, M[3], ALU.subtract)
                        rsl = (slice(CH * cc, CH * (cc + 1)) if last
                               else slice(None))
                        for i, yv in enumerate((y0, y1)):
                            nc.vector.tensor_scalar(
                                ob[:, rsl, :, i],
                                yv.rearrange("p (c h q) -> p (c h) q",
                                             c=nch, q=27),
                                MAGIC, MAGIC, ALU.add, ALU.subtract)
                        if last:
                            nc.sync.dma_start(
                                out=out_d[n, m * 128:(m + 1) * 128,
                                          CH * cc:CH * (cc + 1)],
                                in_=ob[:, rsl].rearrange(
                                    "p h q f -> p h (q f)"))
                    if not last:
                        nc.sync.dma_start(
                            out=out_d[n, m * 128:(m + 1) * 128],
                            in_=ob[:, :, :, :].rearrange(
                                "p h q f -> p h (q f)"))

    nc.compile()
    return nc


def make_in_maps(inputs):
    x = np.ascontiguousarray(np.asarray(inputs["inputVec"], dtype=np.int8))
    w = np.asarray(inputs["weight"], dtype=np.int8)
    scales = np.asarray(inputs["scales"], dtype=np.float32)
    zp = np.asarray(inputs["zeropoints"], dtype=np.int32)
    bias = np.asarray(inputs["bias"], dtype=np.float32)
    assert x.shape == (N, CIN, H, W) and w.shape == (COUT, CIN, KH, KW)

    # host prep: fold per-channel dequant + 0.01 into transformed weights
    wq = (w.astype(np.float64) - zp[:, None, None, None]) \
        * (0.01 * scales.astype(np.float64))[:, None, None, None]
    # U[o,i,r,j] = sum_c G[j,c] wq[o,i,r,c]
    U = np.einsum("jc,oirc->oirj", GMAT, wq)
    # layout [k, cin_part, j, r, m, cout_part]
    Ur = U.reshape(MT, 128, KT, 128, KH, NJ).transpose(2, 3, 5, 4, 0, 1)
    u_h = np.ascontiguousarray(Ur, dtype=np.float16)
    # fold the x-7 per-tensor zp into bias: -7 * 0.01*scale * sum(w-zp)
    w1z = (w.astype(np.float64) - zp[:, None, None, None]).sum(axis=(1, 2, 3))
    b2 = (bias.astype(np.float64)
          - 0.07 * scales.astype(np.float64) * w1z).astype(np.float32)
    return [
        {"x": np.ascontiguousarray(x[c * NPER:(c + 1) * NPER]),
         "u": u_h, "bias2": b2}
        for c in range(NCORES)
    ]


def kernel(**inputs) -> np.ndarray:
    if "nc" not in _CACHE:
        _CACHE["nc"] = _build_program()
    nc = _CACHE["nc"]

    in_maps = make_in_maps(inputs)
    res = run_bass_kernel_spmd(nc, in_maps, list(range(NCORES)))
    out = np.concatenate([res.results[c]["out"] for c in range(NCORES)],
                         axis=0)
    return out
